# revision 28
# baseline (speedup 1.0000x reference)
# Distributed sparse-attention kernel for Trainium2 (8 NeuronCores).
#
# Sharding: core c = (batch b = c//2, head-group g = c%2 of 8 heads).
# Per core, heads are split into two PE partition groups g2 = h%2 (even heads
# on partitions 0-63, odd on 64-127) with ft = h//2 indexing the 4 heads of a
# group.  Attention is computed per 256-query chunk, fused sim->exp->mask->AV:
#   q   = meancenter(x) @ Wc            (LN folded into Wc on host)
#   kv  = [prefix; x] @ Wkv.T           (MQA single head)
#   qn  = q * (8*q_scale*k_scale) * rsqrt(sumsq(q))   (per-query bcast via a
#         block-ones reduce matmul so no partition-broadcast is needed)
#   kn  = raw k; 1/||k|| folded into the Exp activation's per-partition scale
#   P   = exp(kn.T qn * rk) * exp_bias  (bias/causal folded into a
#         multiplicative exp(bias) table, host-packed per 128-key tile)
#   AV  uses va = [v | ones] as stationary so PSUM rows 64-127 hold the
#         softmax denominator pre-broadcast; normalize = reciprocal + mult
#   out = Wo_g.T @ att, summed across the two head-group cores on host.

import numpy as np

B, N, P, DIM, HEADS, DH = 4, 1024, 1024, 1024, 16, 64
HL = 8                 # heads per core
FL = HL * DH           # 512 local q features
J = P + N              # 2048 keys
WIND = 16              # prefix cond-window
QW = 256               # query chunk
NQC = N // QW          # 4 query chunks
CORES = list(range(8))


def _x_units(qch):
    """x-region key tiles for query chunk qch: (ct, qlo, qhi)."""
    q0 = qch * QW
    return [(ct, max(q0, 128 * ct), q0 + QW) for ct in range(2 * qch + 2)]


def _band_units(qch):
    """prefix band tiles: (jt, qlo, qhi, kind)."""
    q0 = qch * QW
    out = [(2 * qch, q0, q0 + 144, "main"),
           (2 * qch + 1, q0 + 128, q0 + 256, "third")]
    if qch > 0:
        out.append((2 * qch - 1, q0, q0 + WIND, "corner"))
    return out


def _bias_layout():
    """Offsets of host-packed bias tiles keyed (qch, g2, ct); emission order
    must match _prep_in_maps' packing order exactly."""
    offs = {}
    off = 0
    for qch in range(NQC):
        for g2 in range(2):
            for ct, qlo, qhi in _x_units(qch):
                w = qhi - qlo
                offs[(qch, g2, ct)] = (off, w)
                off += 128 * 4 * w
    return offs, off


BIAS_OFFS, BIAS_TOTAL = _bias_layout()


def _masks():
    """(q, ft)-major ADDITIVE band masks (0 keep / -1e30 kill)."""
    r = np.arange(128)[:, None]
    t = np.arange(144)[None, :]
    main = np.where((t - r >= 0) & (t - r < WIND), 0.0, -1e30)
    main = np.repeat(main[:, :, None], 4, axis=2).reshape(128, 576)
    tc = np.arange(WIND)[None, :]
    corner = np.where(r - tc >= 128 - WIND + 1, 0.0, -1e30)
    corner = np.repeat(corner[:, :, None], 4, axis=2).reshape(128, 64)
    return main, corner


def _patch_tile_drain():
    """walrus in this image only encodes ~2 sem waits on a CTRL (Drain/Nop)
    instruction; Tile's exit drain attaches every outstanding sem wait to a
    single drain.  Split the waits across extra sync-engine nops."""
    import concourse.tile as tile_mod
    from concourse import mybir
    from concourse.vector_clock import ScopedClock

    if getattr(tile_mod.TileContext, "_drain_split_patch", False):
        return
    MAXW = 1

    _ENGS = {
        mybir.EngineType.PE, mybir.EngineType.Activation,
        mybir.EngineType.Pool, mybir.EngineType.DVE, mybir.EngineType.SP,
    }
    _LIMITS = {}
    _nsplit = [0]
    orig_add = tile_mod.TileContext._add_instruction

    def _add_instruction(self, inst):
        si = inst.sync_info
        lim = _LIMITS.get(inst.engine, 1)
        if (si is not None and si.on_wait and len(si.on_wait) > lim
                and inst.engine in _ENGS):
            waits = list(si.on_wait)
            keep = waits[:lim]
            rest = waits[lim:]
            inst.sync_info = mybir.SyncInfo(
                on_wait=keep, on_update=list(si.on_update or []))
            for i in range(0, len(rest), MAXW):
                _nsplit[0] += 1
                nop = mybir.InstNoOp(
                    name=f"{inst.name}-ws{_nsplit[0]}", ins=[], outs=[])
                nop.engine = inst.engine
                nop.sync_info = mybir.SyncInfo(
                    on_wait=rest[i:i + MAXW], on_update=[])
                orig_add(self, nop)
        orig_add(self, inst)

    tile_mod.TileContext._add_instruction = _add_instruction

    def _drain_and_barrier(self, tick_clock, wait_clock):
        drain_inst = self.nc.sync.drain()
        wait_clock.add_sem_waits(
            drain_inst.ins, ScopedClock({None: tick_clock.global_clock})
        )
        si = drain_inst.ins.sync_info
        waits = list(si.on_wait or []) if si is not None else []
        if len(waits) > MAXW:
            ups = list(si.on_update or []) if si is not None else []
            drain_inst.ins.sync_info = mybir.SyncInfo(on_wait=[], on_update=ups)
            for i in range(0, len(waits), MAXW):
                nop = self.nc.sync.nop(nofuse=True)
                nop.ins.sync_info = mybir.SyncInfo(
                    on_wait=waits[i:i + MAXW], on_update=[])
        self.nc.all_engine_barrier()
        assert self.sems is not None
        popped = self.nc._tile_sem_poison_stack.pop()
        assert popped is self._sem_poison
        self.nc.clear_and_free_semaphores(list(self.sems.allocated().values()))
        self.nc.all_engine_barrier()

    tile_mod.TileContext._drain_and_barrier = _drain_and_barrier
    tile_mod.TileContext._drain_split_patch = True


def _build_nc():
    import ml_dtypes
    import concourse.bass as bass
    import concourse.tile as tile
    from concourse import mybir
    from concourse.alu_op_type import AluOpType

    _patch_tile_drain()

    f32 = mybir.dt.float32
    bf16 = mybir.dt.bfloat16
    bf = ml_dtypes.bfloat16

    nc = bass.Bass("TRN2", target_bir_lowering=False, debug=False)

    xT = nc.dram_tensor("xT", [128, 8 * N], bf16, kind="ExternalInput").ap()
    ctxT = nc.dram_tensor("ctxT", [128, 8 * P], bf16,
                          kind="ExternalInput").ap()
    biasF = nc.dram_tensor("biasF", [BIAS_TOTAL], bf16,
                           kind="ExternalInput").ap()
    wc = nc.dram_tensor("wc", [128, 8 * FL], bf16, kind="ExternalInput").ap()
    wkv = nc.dram_tensor("wkv", [128, 8 * 2 * DH], bf16,
                         kind="ExternalInput").ap()
    wo = nc.dram_tensor("wo", [128, 4 * DIM], bf16, kind="ExternalInput").ap()
    sdkc = nc.dram_tensor("sdkc", [128, 1], f32, kind="ExternalInput").ap()
    outT = nc.dram_tensor("outT", [DIM, N], bf16, kind="ExternalOutput").ap()

    mmain, mcorner = _masks()
    bandm_dram = nc.inline_tensor(mmain.astype(bf), "bandm").ap()
    corner_dram = nc.inline_tensor(mcorner.astype(bf), "cornm").ap()
    idup_np = (np.arange(128)[:, None] % 64 == np.arange(64)[None, :])
    idup_dram = nc.inline_tensor(idup_np.astype(bf), "idup").ap()
    # block-ones: col m sums partitions 64*(m//64) .. +64 -> row-broadcast ssq
    indb_np = (np.arange(128)[:, None] // 64
               == np.arange(128)[None, :] // 64)
    indb_dram = nc.inline_tensor(indb_np.astype(bf), "indb").ap()
    id128_dram = nc.inline_tensor(np.eye(128).astype(bf), "id128").ap()
    ones64_dram = nc.inline_tensor(np.ones((64, 1)).astype(bf), "o64").ap()
    ones164_dram = nc.inline_tensor(np.ones((1, 64), np.float32), "o164").ap()

    Exp = mybir.ActivationFunctionType.Exp
    Ln = mybir.ActivationFunctionType.Ln
    Sq = mybir.ActivationFunctionType.Square
    Cp = mybir.ActivationFunctionType.Copy

    with tile.TileContext(nc) as tc, \
            tc.tile_pool(name="big", bufs=1) as big, \
            tc.tile_pool(name="cst", bufs=1) as cst, \
            tc.tile_pool(name="sqp", bufs=2) as sqp, \
            tc.tile_pool(name="lnp", bufs=2) as lnp, \
            tc.tile_pool(name="bia", bufs=10) as biap, \
            tc.tile_pool(name="ptx", bufs=8) as ptxp, \
            tc.tile_pool(name="rcb", bufs=4) as rcbp, \
            tc.tile_pool(name="osb", bufs=3) as osbp, \
            tc.tile_pool(name="psS", bufs=2, space="PSUM") as psS, \
            tc.tile_pool(name="psA", bufs=2, space="PSUM") as psA:

        # ---- loads ----
        x_sb = big.tile([128, 4, 8, 256], bf16, tag="xT")
        xv = xT.rearrange("p (qb kt n) -> p qb kt n", qb=4, kt=8)
        for qb in range(4):
            nc.sync.dma_start(x_sb[:, qb], xv[:, qb])
        ctx_sb = big.tile([128, 4, 8, 256], bf16, tag="ctxT")

        wc_sb = big.tile([128, 4, 8, 128], bf16, tag="wc")
        wcv = wc.rearrange("p (ft kt f) -> p ft kt f", ft=4, kt=8)
        for ft in range(4):
            nc.scalar.dma_start(wc_sb[:, ft], wcv[:, ft])
        wkv_sb = big.tile([128, 8, 2 * DH], bf16, tag="wkv")
        nc.scalar.dma_start(wkv_sb[:], wkv.rearrange("p (kt f) -> p kt f",
                                                     f=2 * DH))
        idup_sb = cst.tile([128, 64], bf16, tag="idup")
        nc.scalar.dma_start(idup_sb[:], idup_dram)
        indb_sb = cst.tile([128, 128], bf16, tag="indb")
        nc.scalar.dma_start(indb_sb[:], indb_dram)
        sdk_sb = cst.tile([128, 1], f32, tag="sdk")
        nc.scalar.dma_start(sdk_sb[:], sdkc)
        bandm_sb = cst.tile([128, 576], bf16, tag="bandm")
        nc.scalar.dma_start(bandm_sb[:], bandm_dram)
        corner_sb = cst.tile([128, 64], bf16, tag="cornm")
        nc.scalar.dma_start(corner_sb[:], corner_dram)
        id128_sb = cst.tile([128, 128], bf16, tag="id128")
        nc.scalar.dma_start(id128_sb[:], id128_dram)
        o64_sb = cst.tile([64, 1], bf16, tag="o64")
        nc.scalar.dma_start(o64_sb[:], ones64_dram)
        o164_sb = cst.tile([1, 64], f32, tag="o164")
        nc.scalar.dma_start(o164_sb[:], ones164_dram)
        rkrow_sb = cst.tile([1, J], mybir.dt.float32r, tag="rkrow")
        wo_sb = big.tile([128, 4, DIM], bf16, tag="wo")

        eps_sb = cst.tile([128, 1], f32, tag="eps")
        nc.vector.memset(eps_sb[:], 1e-24)
        tblw_sb = cst.tile([128, 1], f32, tag="tblw")
        nc.scalar.activation(tblw_sb[:], eps_sb[:], Exp)
        garb_sb = cst.tile([128, 512], bf16, tag="garb")
        nc.vector.memset(garb_sb[:], 0.125)

        kvT_sb = big.tile([128, J], bf16, tag="kvT")   # k rows 0-63, v 64-127
        knb_sb = big.tile([128, J], bf16, tag="knb")   # l2norm'd k, dup'd
        qn_sb = big.tile([128, NQC, QW, 4], bf16, tag="qn")
        att_sb = big.tile([128, 4, N], bf16, tag="att")
        va_sb = big.tile([128, 16, 128], bf16, tag="va")  # [v | ones]
        nc.vector.memset(va_sb[:, :, DH:128], 1.0)

        # ---- PE warmup during loads ----
        for i in range(16):
            wps = psS.tile([128, 1024], f32, tag="S", name=f"warm{i}")
            nc.tensor.matmul(wps[:, 0:512], lhsT=garb_sb[:, 0:128],
                             rhs=garb_sb[:], start=True, stop=True)

        # ---- q projection + normalize (deferred one ft for overlap) ----
        def emit_q_mm(ft):
            ps = psS.tile([128, 1024], f32, tag="S", name=f"qps{ft}")
            for qb in range(4):
                for kt in range(8):
                    nc.tensor.matmul(
                        ps[:, qb * 256:(qb + 1) * 256],
                        lhsT=wc_sb[:, ft, kt, :],
                        rhs=x_sb[:, qb, kt, :],
                        start=(kt == 0), stop=(kt == 7))
            return ps

        def emit_q_norm(ft, ps):
            qf = lnp.tile([128, 1024], bf16, tag="qf", name=f"qf{ft}")
            nc.vector.tensor_copy(out=qf[:], in_=ps[:])
            sq = sqp.tile([128, 1024], bf16, tag="sq", name=f"qsq{ft}")
            nc.vector.tensor_mul(sq[:], qf[:], qf[:])
            ssq = psA.tile([128, 1024], f32, tag="avg", name=f"qssq{ft}")
            for h in range(2):
                nc.tensor.matmul(ssq[:, h * 512:(h + 1) * 512],
                                 lhsT=indb_sb[:],
                                 rhs=sq[:, h * 512:(h + 1) * 512],
                                 start=True, stop=True)
            lnq = lnp.tile([128, 1024], f32, tag="ln", name=f"qln{ft}")
            nc.scalar.activation(lnq[:], ssq[:], Ln, bias=eps_sb[:])
            rqb = lnp.tile([128, 1024], f32, tag="rqb", name=f"qrq{ft}")
            nc.scalar.activation(rqb[:], lnq[:], Exp, scale=-0.5)
            # qn = (q * sdk) * rsqrt(ssq), scattered to (qch, q, ft) layout
            nc.vector.scalar_tensor_tensor(
                out=qn_sb[:, :, :, ft],
                in0=qf[:].rearrange("p (c x) -> p c x", x=QW),
                scalar=sdk_sb[:],
                in1=rqb[:].rearrange("p (c x) -> p c x", x=QW),
                op0=AluOpType.mult, op1=AluOpType.mult)

        def emit_kv_mm(jh):
            src = ctx_sb if jh == 0 else x_sb
            ps = psS.tile([128, 1024], f32, tag="S", name=f"kvps{jh}")
            for qb in range(4):
                for kt in range(8):
                    nc.tensor.matmul(
                        ps[:, qb * 256:(qb + 1) * 256],
                        lhsT=wkv_sb[:, kt, :],
                        rhs=src[:, qb, kt, :],
                        start=(kt == 0), stop=(kt == 7))
            js = slice(jh * 1024, (jh + 1) * 1024)
            nc.vector.tensor_copy(out=kvT_sb[:, js], in_=ps[:])
            kf = sqp.tile([64, 1024], bf16, tag="kf", name=f"kf{jh}")
            nc.vector.tensor_copy(out=kf[:], in_=ps[0:64, :])
            ksq = sqp.tile([64, 1024], bf16, tag="ksq", name=f"ksq{jh}")
            nc.vector.tensor_mul(ksq[:], kf[:], kf[:])
            ssr = psA.tile([128, 1024], f32, tag="avg", name=f"kssr{jh}")
            for h in range(2):
                nc.tensor.matmul(ssr[0:1, h * 512:(h + 1) * 512],
                                 lhsT=o64_sb[:],
                                 rhs=ksq[:, h * 512:(h + 1) * 512],
                                 start=True, stop=True)
            lnr = lnp.tile([1, 1024], f32, tag="lnr", name=f"klnr{jh}")
            nc.scalar.activation(lnr[:], ssr[0:1, :], Ln, bias=eps_sb[0:1])
            nc.scalar.activation(rkrow_sb[0:1, js], lnr[:], Exp, scale=-0.5)
            return js

        def emit_kv_norm(jh, js):
            rkb = psA.tile([128, 1024], f32, tag="avg", name=f"krkb{jh}")
            for h in range(2):
                nc.tensor.matmul(
                    rkb[0:64, h * 512:(h + 1) * 512],
                    lhsT=o164_sb[:].bitcast(mybir.dt.float32r),
                    rhs=rkrow_sb[0:1, jh * 1024 + h * 512:
                                  jh * 1024 + (h + 1) * 512],
                    start=True, stop=True)
            nc.vector.tensor_mul(knb_sb[0:64, js], kvT_sb[0:64, js],
                                 rkb[0:64, :])
            nc.vector.tensor_copy(out=knb_sb[64:128, js],
                                  in_=knb_sb[0:64, js])

        def emit_ktr(jh):
            """transpose v tiles of half jh into va."""
            js0 = jh * 1024
            vtps = psA.tile([128, 1024], f32, tag="avg", name=f"vtp{jh}")
            vtv = vtps[:, 0:256].bitcast(bf16)
            for i in range(8):
                nc.tensor.transpose(
                    vtv[:, i * 64:(i + 1) * 64],
                    kvT_sb[64:128, js0 + i * 128:js0 + (i + 1) * 128],
                    idup_sb[64:128, :])
            nc.vector.tensor_copy(
                out=va_sb[:, jh * 8:(jh + 1) * 8, 0:DH],
                in_=vtv[:].rearrange("p (t d) -> p t d", d=64))

        qpss = {}
        qpss[0] = emit_q_mm(0)
        qpss[1] = emit_q_mm(1)
        ctxv = ctxT.rearrange("p (qb kt n) -> p qb kt n", qb=4, kt=8)
        for qb in range(4):
            nc.gpsimd.dma_start(ctx_sb[:, qb], ctxv[:, qb])
        nc.gpsimd.dma_start(wo_sb[:], wo.rearrange("p (ft e) -> p ft e",
                                                   e=DIM))
        kv1 = emit_kv_mm(1)
        emit_q_norm(0, qpss[0])
        qpss[2] = emit_q_mm(2)
        emit_kv_norm(1, kv1)
        emit_q_norm(1, qpss[1])
        kv0 = emit_kv_mm(0)
        emit_ktr(1)
        emit_q_norm(2, qpss[2])
        qpss[3] = emit_q_mm(3)
        emit_kv_norm(0, kv0)
        emit_q_norm(3, qpss[3])
        emit_ktr(0)

        # ---- fused attention per query chunk ----
        def emit_sim(qch, g2, jt, qlo, qhi, kind, ct=None):
            """sim matmuls + exp(scale=1/||k||) + bias/mask mult -> ptx."""
            w = qhi - qlo
            base = 64 * g2
            lhs = knb_sb
            q0 = qch * QW
            w4 = 4 * w
            ps = psS.tile([128, 1024], f32, tag="S",
                          name=f"sps{qch}{g2}{jt}{kind}")
            qv = qn_sb[base:base + 64, :, :, :].rearrange(
                "p c x f -> p (c x f)")
            if kind == "x":
                off, bw = BIAS_OFFS[(qch, g2, ct)]
                assert bw == w
                bt = biap.tile([128, 1024], bf16, tag="bias",
                               name=f"bt{qch}{g2}{jt}")
                nc.sync.dma_start(
                    bt[:, 0:w4],
                    biasF[off:off + 128 * w4].rearrange("(p x) -> p x", p=128))
                add = bt
            else:
                add = corner_sb if kind == "corner" else bandm_sb
            for lo in range(0, w4, 512):
                hi = min(w4, lo + 512)
                nc.tensor.matmul(
                    ps[:, lo:hi],
                    lhsT=lhs[base:base + 64, jt * 128:(jt + 1) * 128],
                    rhs=qv[:, 4 * qlo + lo:4 * qlo + hi],
                    start=True, stop=False)
            for lo in range(0, w4, 512):
                hi = min(w4, lo + 512)
                nc.tensor.matmul(ps[:, lo:hi], lhsT=id128_sb[:],
                                 rhs=add[:, lo:hi], start=False, stop=True)
            pt = ptxp.tile([128, 1024], bf16, tag="ptx",
                           name=f"ptx{qch}{g2}{jt}{kind}")
            nc.scalar.activation(pt[:, 0:w4], ps[:, 0:w4], Exp)
            return pt

        def emit_av(avps, q0, jt, qlo, qhi, pt, start, stop, pt0=None):
            c0 = 4 * (qlo - q0)
            pt0 = 0 if pt0 is None else 4 * pt0
            cuts = sorted({c0, 4 * (qhi - q0)}
                          | {b for b in (512,) if c0 < b < 4 * (qhi - q0)})
            for lo, hi in zip(cuts[:-1], cuts[1:]):
                nc.tensor.matmul(avps[:, lo:hi],
                                 lhsT=va_sb[:, jt, :],
                                 rhs=pt[:, pt0 + lo - c0:pt0 + hi - c0],
                                 start=start, stop=stop)

        def emit_attnorm(avps, qch, g2):
            q0 = qch * QW
            lnd = rcbp.tile([64, 1024], f32, tag="lnd",
                            name=f"lnd{qch}{g2}")
            nc.scalar.activation(lnd[:], avps[64:128, :], Ln)
            rb = rcbp.tile([64, 1024], f32, tag="rcb",
                           name=f"rcb{qch}{g2}")
            nc.scalar.activation(rb[:], lnd[:], Exp, scale=-1.0)
            nc.vector.tensor_mul(
                att_sb[64 * g2:64 * g2 + 64, :, q0:q0 + QW],
                avps[0:64, :].rearrange("p (x f) -> p f x", f=4),
                rb[:].rearrange("p (x f) -> p f x", f=4))

        def emit_outproj(qc, ets=range(8)):
            for et in ets:
                ops = psS.tile([128, 1024], f32, tag="S", name=f"op{qc}{et}")
                for ftile in range(4):
                    nc.tensor.matmul(
                        ops[:, 0:512],
                        lhsT=wo_sb[:, ftile, et * 128:(et + 1) * 128],
                        rhs=att_sb[:, ftile, qc * 512:(qc + 1) * 512],
                        start=(ftile == 0), stop=(ftile == 3))
                o = osbp.tile([128, 512], bf16, tag="osb", name=f"o{qc}{et}")
                nc.vector.tensor_copy(out=o[:], in_=ops[:, 0:512])
                nc.gpsimd.dma_start(
                    out=outT[et * 128:(et + 1) * 128,
                             qc * 512:(qc + 1) * 512],
                    in_=o[:])

        deferred = {0: [], 1: []}
        order = [3, 2, 1, 0]
        for oi, qch in enumerate(order):
            q0 = qch * QW
            for g2 in range(2):
                avps = None
                xs = _x_units(qch)
                units = ([("x",) + xs[0]]
                         + [("b", jt, lo, hi, kk)
                            for jt, lo, hi, kk in _band_units(qch)]
                         + [("x",) + u for u in xs[1:]])
                pend = None
                for i, u in enumerate(units):
                    if u[0] == "x":
                        _, ct, qlo, qhi = u
                        jt, kind = 8 + ct, "x"
                        pt = emit_sim(qch, g2, jt, qlo, qhi, kind, ct=ct)
                    else:
                        _, jt, qlo, qhi, kind = u
                        pt = emit_sim(qch, g2, jt, qlo, qhi, kind)
                    # run the previous chunk's normalize/out-proj behind this
                    # group's first sim so the normalize chain and the pool
                    # WAR on the av tile are off the PE critical path
                    if i == 0 and deferred[g2]:
                        for fn in deferred[g2]:
                            fn()
                        deferred[g2] = []
                    if oi == 3 and g2 == 1 and i == 0:
                        for fn in deferred[0]:
                            fn()
                        deferred[0] = []
                    if pend is not None:
                        if avps is None:
                            avps = psA.tile([128, 1024], f32, tag="avg",
                                            name=f"av{qch}{g2}")
                        emit_av(avps, q0, *pend, start=(i == 1), stop=False)
                    pend = (jt, qlo, qhi, pt)
                emit_av(avps, q0, *pend, start=False, stop=True)
                deferred[g2] = [
                    lambda a=avps, q=qch, g=g2: emit_attnorm(a, q, g)]
                if oi == 1 and g2 == 1:
                    deferred[g2].append(lambda: emit_outproj(1, range(0, 4)))
                if oi == 2 and g2 == 0:
                    deferred[g2].append(lambda: emit_outproj(1, range(4, 8)))
        for g2 in range(2):
            for fn in deferred[g2]:
                fn()
        emit_outproj(0)

    return nc


_NC = None


def _get_nc():
    global _NC
    if _NC is None:
        _NC = _build_nc()
    return _NC


def _prep_in_maps(x, prefix_context, attn_bias, gamma, Wq, Wkv, q_scale,
                  k_scale, Wo, mask):
    import ml_dtypes
    bf = ml_dtypes.bfloat16

    x = np.asarray(x, np.float32)
    prefix_context = np.asarray(prefix_context, np.float32)
    attn_bias = np.asarray(attn_bias, np.float32)
    gamma = np.asarray(gamma, np.float32)
    Wq = np.asarray(Wq, np.float32)
    Wkv = np.asarray(Wkv, np.float32)
    q_scale = np.asarray(q_scale, np.float32)
    k_scale = np.asarray(k_scale, np.float32)
    Wo = np.asarray(Wo, np.float32)
    mask = np.asarray(mask)

    tril = np.triu(np.ones((N, N), np.float32))  # keep key c <= query i [c, i]
    sdk = (8.0 * q_scale * k_scale).astype(np.float32)
    sdkc = np.tile(sdk, 2).reshape(128, 1)
    wkvT = np.ascontiguousarray(
        Wkv.T.reshape(8, 128, 2 * DH).transpose(1, 0, 2).reshape(
            128, 8 * 2 * DH)).astype(bf)

    in_maps = []
    for c in CORES:
        b, g = c // 2, c % 2
        hs = slice(g * HL, (g + 1) * HL)
        # additive bias with causal/key-mask kills at -1e30, [h, c, i]
        eb = np.where(tril[None] > 0, attn_bias[hs].transpose(0, 2, 1),
                      -1e30)
        maskf = mask[b]
        if not maskf.all():
            eb[:, ~maskf, :] = -1e30
        eb = eb.astype(bf)
        # pack bias tiles in kernel consumption order
        bflat = np.empty(BIAS_TOTAL, bf)
        for qch in range(NQC):
            for g2 in range(2):
                heads = [2 * ft + g2 for ft in range(4)]
                for ct, qlo, qhi in _x_units(qch):
                    off, w = BIAS_OFFS[(qch, g2, ct)]
                    t = eb[heads, 128 * ct:128 * (ct + 1), qlo:qhi]
                    bflat[off:off + 128 * 4 * w] = (
                        t.transpose(1, 2, 0).ravel())
        Wg = Wq[g * FL:(g + 1) * FL] * gamma[None, :]
        s = Wg.sum(axis=1)
        wcT = Wg.T - s[None, :] / DIM
        wog = Wo[:, g * FL:(g + 1) * FL]
        def pmaj(a, nt):
            # [nt*128, m] -> [128, nt*m] partition-major
            m = a.shape[1]
            return np.ascontiguousarray(
                a.reshape(nt, 128, m).transpose(1, 0, 2).reshape(128, nt * m))

        def qbmaj(a):
            # [1024, 1024] -> [128, (qb, kt, 256)]
            return np.ascontiguousarray(
                a.reshape(8, 128, 4, 256).transpose(1, 2, 0, 3).reshape(
                    128, 8192))

        wcf = np.ascontiguousarray(
            wcT.reshape(8, 128, 4, 128).transpose(1, 2, 0, 3).reshape(
                128, 4096))
        in_maps.append(dict(
            xT=qbmaj(x[b].T).astype(bf),
            ctxT=qbmaj(prefix_context[b].T).astype(bf),
            biasF=bflat,
            wc=wcf.astype(bf),
            wkv=wkvT,
            wo=pmaj(wog.T, 4).astype(bf),
            sdkc=sdkc,
        ))
    return in_maps


def kernel(**inputs):
    from concourse.bass_utils import run_bass_kernel_spmd

    nc = _get_nc()
    in_maps = _prep_in_maps(**inputs)
    res = run_bass_kernel_spmd(nc, in_maps, CORES).results
    out = np.empty((B, N, DIM), np.float32)
    for b in range(B):
        out[b] = (np.asarray(res[2 * b]["outT"]).astype(np.float32)
                  + np.asarray(res[2 * b + 1]["outT"]).astype(np.float32)).T
    return out


# revision 29
# speedup vs baseline: 1.0443x; 1.0443x over previous
# Distributed sparse-attention kernel for Trainium2 (8 NeuronCores).
#
# Sharding: core c = (batch b = c//2, head-group g = c%2 of 8 heads).
# Per core, heads are split into two PE partition groups g2 = h%2 (even heads
# on partitions 0-63, odd on 64-127) with ft = h//2 indexing the 4 heads of a
# group.  Attention is computed per 256-query chunk, fused sim->exp->mask->AV:
#   q   = meancenter(x) @ Wc            (LN folded into Wc on host)
#   kv  = [prefix; x] @ Wkv.T           (MQA single head)
#   qn  = q * (8*q_scale*k_scale) * rsqrt(sumsq(q))   (per-query bcast via a
#         block-ones reduce matmul so no partition-broadcast is needed)
#   kn  = raw k; 1/||k|| folded into the Exp activation's per-partition scale
#   P   = exp(kn.T qn * rk) * exp_bias  (bias/causal folded into a
#         multiplicative exp(bias) table, host-packed per 128-key tile)
#   AV  uses va = [v | ones] as stationary so PSUM rows 64-127 hold the
#         softmax denominator pre-broadcast; normalize = reciprocal + mult
#   out = Wo_g.T @ att, summed across the two head-group cores on host.

import numpy as np

B, N, P, DIM, HEADS, DH = 4, 1024, 1024, 1024, 16, 64
HL = 8                 # heads per core
FL = HL * DH           # 512 local q features
J = P + N              # 2048 keys
WIND = 16              # prefix cond-window
QW = 256               # query chunk
NQC = N // QW          # 4 query chunks
CORES = list(range(8))


def _x_units(qch):
    """x-region key tiles for query chunk qch: (ct, qlo, qhi)."""
    q0 = qch * QW
    return [(ct, max(q0, 128 * ct), q0 + QW) for ct in range(2 * qch + 2)]


def _band_units(qch):
    """prefix band tiles: (jt, qlo, qhi, kind)."""
    q0 = qch * QW
    out = [(2 * qch, q0, q0 + 144, "main"),
           (2 * qch + 1, q0 + 128, q0 + 256, "third")]
    if qch > 0:
        out.append((2 * qch - 1, q0, q0 + WIND, "corner"))
    return out


def _bias_layout():
    """Offsets of host-packed bias tiles keyed (qch, g2, ct); emission order
    must match _prep_in_maps' packing order exactly."""
    offs = {}
    off = 0
    for qch in range(NQC):
        for g2 in range(2):
            for ct, qlo, qhi in _x_units(qch):
                w = qhi - qlo
                offs[(qch, g2, ct)] = (off, w)
                off += 128 * 4 * w
    return offs, off


BIAS_OFFS, BIAS_TOTAL = _bias_layout()


def _masks():
    """(q, ft)-major ADDITIVE band masks (0 keep / -1e30 kill)."""
    r = np.arange(128)[:, None]
    t = np.arange(144)[None, :]
    main = np.where((t - r >= 0) & (t - r < WIND), 0.0, -1e30)
    main = np.repeat(main[:, :, None], 4, axis=2).reshape(128, 576)
    tc = np.arange(WIND)[None, :]
    corner = np.where(r - tc >= 128 - WIND + 1, 0.0, -1e30)
    corner = np.repeat(corner[:, :, None], 4, axis=2).reshape(128, 64)
    return main, corner


def _patch_tile_drain():
    """walrus in this image only encodes ~2 sem waits on a CTRL (Drain/Nop)
    instruction; Tile's exit drain attaches every outstanding sem wait to a
    single drain.  Split the waits across extra sync-engine nops."""
    import concourse.tile as tile_mod
    from concourse import mybir
    from concourse.vector_clock import ScopedClock

    if getattr(tile_mod.TileContext, "_drain_split_patch", False):
        return
    MAXW = 1

    _ENGS = {
        mybir.EngineType.PE, mybir.EngineType.Activation,
        mybir.EngineType.Pool, mybir.EngineType.DVE, mybir.EngineType.SP,
    }
    _LIMITS = {}
    _nsplit = [0]
    orig_add = tile_mod.TileContext._add_instruction

    def _add_instruction(self, inst):
        si = inst.sync_info
        lim = _LIMITS.get(inst.engine, 1)
        if (si is not None and si.on_wait and len(si.on_wait) > lim
                and inst.engine in _ENGS):
            waits = list(si.on_wait)
            keep = waits[:lim]
            rest = waits[lim:]
            inst.sync_info = mybir.SyncInfo(
                on_wait=keep, on_update=list(si.on_update or []))
            for i in range(0, len(rest), MAXW):
                _nsplit[0] += 1
                nop = mybir.InstNoOp(
                    name=f"{inst.name}-ws{_nsplit[0]}", ins=[], outs=[])
                nop.engine = inst.engine
                nop.sync_info = mybir.SyncInfo(
                    on_wait=rest[i:i + MAXW], on_update=[])
                orig_add(self, nop)
        orig_add(self, inst)

    tile_mod.TileContext._add_instruction = _add_instruction

    def _drain_and_barrier(self, tick_clock, wait_clock):
        drain_inst = self.nc.sync.drain()
        wait_clock.add_sem_waits(
            drain_inst.ins, ScopedClock({None: tick_clock.global_clock})
        )
        si = drain_inst.ins.sync_info
        waits = list(si.on_wait or []) if si is not None else []
        if len(waits) > MAXW:
            ups = list(si.on_update or []) if si is not None else []
            drain_inst.ins.sync_info = mybir.SyncInfo(on_wait=[], on_update=ups)
            for i in range(0, len(waits), MAXW):
                nop = self.nc.sync.nop(nofuse=True)
                nop.ins.sync_info = mybir.SyncInfo(
                    on_wait=waits[i:i + MAXW], on_update=[])
        self.nc.all_engine_barrier()
        assert self.sems is not None
        popped = self.nc._tile_sem_poison_stack.pop()
        assert popped is self._sem_poison
        self.nc.clear_and_free_semaphores(list(self.sems.allocated().values()))
        self.nc.all_engine_barrier()

    tile_mod.TileContext._drain_and_barrier = _drain_and_barrier
    tile_mod.TileContext._drain_split_patch = True


def _build_nc():
    import ml_dtypes
    import concourse.bass as bass
    import concourse.tile as tile
    from concourse import mybir
    from concourse.alu_op_type import AluOpType

    _patch_tile_drain()

    f32 = mybir.dt.float32
    bf16 = mybir.dt.bfloat16
    bf = ml_dtypes.bfloat16

    nc = bass.Bass("TRN2", target_bir_lowering=False, debug=False)

    xT = nc.dram_tensor("xT", [128, 8 * N], bf16, kind="ExternalInput").ap()
    ctxT = nc.dram_tensor("ctxT", [128, 8 * P], bf16,
                          kind="ExternalInput").ap()
    biasF = nc.dram_tensor("biasF", [BIAS_TOTAL], bf16,
                           kind="ExternalInput").ap()
    wc = nc.dram_tensor("wc", [128, 8 * FL], bf16, kind="ExternalInput").ap()
    wkv = nc.dram_tensor("wkv", [128, 8 * 2 * DH], bf16,
                         kind="ExternalInput").ap()
    wo = nc.dram_tensor("wo", [128, 4 * DIM], bf16, kind="ExternalInput").ap()
    sdkc = nc.dram_tensor("sdkc", [128, 1], f32, kind="ExternalInput").ap()
    outT = nc.dram_tensor("outT", [DIM, N], bf16, kind="ExternalOutput").ap()

    mmain, mcorner = _masks()
    bandm_dram = nc.inline_tensor(mmain.astype(bf), "bandm").ap()
    corner_dram = nc.inline_tensor(mcorner.astype(bf), "cornm").ap()
    idup_np = (np.arange(128)[:, None] % 64 == np.arange(64)[None, :])
    idup_dram = nc.inline_tensor(idup_np.astype(bf), "idup").ap()
    # block-ones: col m sums partitions 64*(m//64) .. +64 -> row-broadcast ssq
    indb_np = (np.arange(128)[:, None] // 64
               == np.arange(128)[None, :] // 64)
    indb_dram = nc.inline_tensor(indb_np.astype(bf), "indb").ap()
    id128_dram = nc.inline_tensor(np.eye(128).astype(bf), "id128").ap()
    ones64_dram = nc.inline_tensor(np.ones((64, 1)).astype(bf), "o64").ap()
    ones164_dram = nc.inline_tensor(np.ones((1, 64), np.float32), "o164").ap()

    Exp = mybir.ActivationFunctionType.Exp
    Ln = mybir.ActivationFunctionType.Ln
    Sq = mybir.ActivationFunctionType.Square
    Cp = mybir.ActivationFunctionType.Copy

    with tile.TileContext(nc) as tc, \
            tc.tile_pool(name="big", bufs=1) as big, \
            tc.tile_pool(name="cst", bufs=1) as cst, \
            tc.tile_pool(name="sqp", bufs=2) as sqp, \
            tc.tile_pool(name="lnp", bufs=2) as lnp, \
            tc.tile_pool(name="bia", bufs=10) as biap, \
            tc.tile_pool(name="ptx", bufs=8) as ptxp, \
            tc.tile_pool(name="rcb", bufs=4) as rcbp, \
            tc.tile_pool(name="osb", bufs=3) as osbp, \
            tc.tile_pool(name="psS", bufs=2, space="PSUM") as psS, \
            tc.tile_pool(name="psA", bufs=2, space="PSUM") as psA:

        # ---- loads ----
        x_sb = big.tile([128, 4, 8, 256], bf16, tag="xT")
        xv = xT.rearrange("p (qb kt n) -> p qb kt n", qb=4, kt=8)
        for qb in range(4):
            nc.sync.dma_start(x_sb[:, qb], xv[:, qb])
        ctx_sb = big.tile([128, 4, 8, 256], bf16, tag="ctxT")
        ctxv = ctxT.rearrange("p (qb kt n) -> p qb kt n", qb=4, kt=8)
        for qb in range(4):
            nc.sync.dma_start(ctx_sb[:, qb], ctxv[:, qb])
        wo_sb = big.tile([128, 4, DIM], bf16, tag="wo")
        nc.sync.dma_start(wo_sb[:], wo.rearrange("p (ft e) -> p ft e",
                                                 e=DIM))

        wc_sb = big.tile([128, 4, 8, 128], bf16, tag="wc")
        wcv = wc.rearrange("p (ft kt f) -> p ft kt f", ft=4, kt=8)
        for ft in range(4):
            nc.scalar.dma_start(wc_sb[:, ft], wcv[:, ft])
        wkv_sb = big.tile([128, 8, 2 * DH], bf16, tag="wkv")
        nc.scalar.dma_start(wkv_sb[:], wkv.rearrange("p (kt f) -> p kt f",
                                                     f=2 * DH))
        idup_sb = cst.tile([128, 64], bf16, tag="idup")
        nc.scalar.dma_start(idup_sb[:], idup_dram)
        indb_sb = cst.tile([128, 128], bf16, tag="indb")
        nc.scalar.dma_start(indb_sb[:], indb_dram)
        sdk_sb = cst.tile([128, 1], f32, tag="sdk")
        nc.scalar.dma_start(sdk_sb[:], sdkc)
        bandm_sb = cst.tile([128, 576], bf16, tag="bandm")
        nc.scalar.dma_start(bandm_sb[:], bandm_dram)
        corner_sb = cst.tile([128, 64], bf16, tag="cornm")
        nc.scalar.dma_start(corner_sb[:], corner_dram)
        id128_sb = cst.tile([128, 128], bf16, tag="id128")
        nc.scalar.dma_start(id128_sb[:], id128_dram)
        o64_sb = cst.tile([64, 1], bf16, tag="o64")
        nc.scalar.dma_start(o64_sb[:], ones64_dram)
        o164_sb = cst.tile([1, 64], f32, tag="o164")
        nc.scalar.dma_start(o164_sb[:], ones164_dram)
        rkrow_sb = cst.tile([1, J], mybir.dt.float32r, tag="rkrow")

        eps_sb = cst.tile([128, 1], f32, tag="eps")
        nc.vector.memset(eps_sb[:], 1e-24)
        tblw_sb = cst.tile([128, 1], f32, tag="tblw")
        nc.scalar.activation(tblw_sb[:], eps_sb[:], Exp)
        garb_sb = cst.tile([128, 512], bf16, tag="garb")
        nc.vector.memset(garb_sb[:], 0.125)

        kvT_sb = big.tile([128, J], bf16, tag="kvT")   # k rows 0-63, v 64-127
        knb_sb = big.tile([128, J], bf16, tag="knb")   # l2norm'd k, dup'd
        qn_sb = big.tile([128, NQC, QW, 4], bf16, tag="qn")
        att_sb = big.tile([128, 4, N], bf16, tag="att")
        va_sb = big.tile([128, 16, 128], bf16, tag="va")  # [v | ones]
        nc.vector.memset(va_sb[:, :, DH:128], 1.0)

        # ---- PE warmup during loads ----
        for i in range(16):
            wps = psS.tile([128, 1024], f32, tag="S", name=f"warm{i}")
            nc.tensor.matmul(wps[:, 0:512], lhsT=garb_sb[:, 0:128],
                             rhs=garb_sb[:], start=True, stop=True)

        # ---- q projection + normalize (deferred one ft for overlap) ----
        def emit_q_mm(ft):
            ps = psS.tile([128, 1024], f32, tag="S", name=f"qps{ft}")
            for qb in range(4):
                for kt in range(8):
                    nc.tensor.matmul(
                        ps[:, qb * 256:(qb + 1) * 256],
                        lhsT=wc_sb[:, ft, kt, :],
                        rhs=x_sb[:, qb, kt, :],
                        start=(kt == 0), stop=(kt == 7))
            return ps

        def emit_q_norm(ft, ps):
            qf = lnp.tile([128, 1024], bf16, tag="qf", name=f"qf{ft}")
            nc.vector.tensor_copy(out=qf[:], in_=ps[:])
            sq = sqp.tile([128, 1024], bf16, tag="sq", name=f"qsq{ft}")
            nc.vector.tensor_mul(sq[:], qf[:], qf[:])
            ssq = psA.tile([128, 1024], f32, tag="avg", name=f"qssq{ft}")
            for h in range(2):
                nc.tensor.matmul(ssq[:, h * 512:(h + 1) * 512],
                                 lhsT=indb_sb[:],
                                 rhs=sq[:, h * 512:(h + 1) * 512],
                                 start=True, stop=True)
            lnq = lnp.tile([128, 1024], f32, tag="ln", name=f"qln{ft}")
            nc.scalar.activation(lnq[:], ssq[:], Ln, bias=eps_sb[:])
            rqb = lnp.tile([128, 1024], f32, tag="rqb", name=f"qrq{ft}")
            nc.scalar.activation(rqb[:], lnq[:], Exp, scale=-0.5)
            # qn = (q * sdk) * rsqrt(ssq), scattered to (qch, q, ft) layout
            nc.vector.scalar_tensor_tensor(
                out=qn_sb[:, :, :, ft],
                in0=qf[:].rearrange("p (c x) -> p c x", x=QW),
                scalar=sdk_sb[:],
                in1=rqb[:].rearrange("p (c x) -> p c x", x=QW),
                op0=AluOpType.mult, op1=AluOpType.mult)

        def emit_kv_mm(jh):
            src = ctx_sb if jh == 0 else x_sb
            ps = psS.tile([128, 1024], f32, tag="S", name=f"kvps{jh}")
            for qb in range(4):
                for kt in range(8):
                    nc.tensor.matmul(
                        ps[:, qb * 256:(qb + 1) * 256],
                        lhsT=wkv_sb[:, kt, :],
                        rhs=src[:, qb, kt, :],
                        start=(kt == 0), stop=(kt == 7))
            js = slice(jh * 1024, (jh + 1) * 1024)
            nc.vector.tensor_copy(out=kvT_sb[:, js], in_=ps[:])
            kf = sqp.tile([64, 1024], bf16, tag="kf", name=f"kf{jh}")
            nc.vector.tensor_copy(out=kf[:], in_=ps[0:64, :])
            ksq = sqp.tile([64, 1024], bf16, tag="ksq", name=f"ksq{jh}")
            nc.vector.tensor_mul(ksq[:], kf[:], kf[:])
            ssr = psA.tile([128, 1024], f32, tag="avg", name=f"kssr{jh}")
            for h in range(2):
                nc.tensor.matmul(ssr[0:1, h * 512:(h + 1) * 512],
                                 lhsT=o64_sb[:],
                                 rhs=ksq[:, h * 512:(h + 1) * 512],
                                 start=True, stop=True)
            lnr = lnp.tile([1, 1024], f32, tag="lnr", name=f"klnr{jh}")
            nc.scalar.activation(lnr[:], ssr[0:1, :], Ln, bias=eps_sb[0:1])
            nc.scalar.activation(rkrow_sb[0:1, js], lnr[:], Exp, scale=-0.5)
            return js

        def emit_kv_norm(jh, js):
            rkb = psA.tile([128, 1024], f32, tag="avg", name=f"krkb{jh}")
            for h in range(2):
                nc.tensor.matmul(
                    rkb[0:64, h * 512:(h + 1) * 512],
                    lhsT=o164_sb[:].bitcast(mybir.dt.float32r),
                    rhs=rkrow_sb[0:1, jh * 1024 + h * 512:
                                  jh * 1024 + (h + 1) * 512],
                    start=True, stop=True)
            nc.vector.tensor_mul(knb_sb[0:64, js], kvT_sb[0:64, js],
                                 rkb[0:64, :])
            nc.vector.tensor_copy(out=knb_sb[64:128, js],
                                  in_=knb_sb[0:64, js])

        def emit_ktr(jh):
            """transpose v tiles of half jh into va."""
            js0 = jh * 1024
            vtps = psA.tile([128, 1024], f32, tag="avg", name=f"vtp{jh}")
            vtv = vtps[:, 0:256].bitcast(bf16)
            for i in range(8):
                nc.tensor.transpose(
                    vtv[:, i * 64:(i + 1) * 64],
                    kvT_sb[64:128, js0 + i * 128:js0 + (i + 1) * 128],
                    idup_sb[64:128, :])
            nc.vector.tensor_copy(
                out=va_sb[:, jh * 8:(jh + 1) * 8, 0:DH],
                in_=vtv[:].rearrange("p (t d) -> p t d", d=64))

        qpss = {}
        qpss[0] = emit_q_mm(0)
        qpss[1] = emit_q_mm(1)
        kv1 = emit_kv_mm(1)
        emit_q_norm(0, qpss[0])
        qpss[2] = emit_q_mm(2)
        emit_kv_norm(1, kv1)
        emit_q_norm(1, qpss[1])
        kv0 = emit_kv_mm(0)
        emit_ktr(1)
        emit_q_norm(2, qpss[2])
        qpss[3] = emit_q_mm(3)
        emit_kv_norm(0, kv0)
        emit_q_norm(3, qpss[3])
        emit_ktr(0)

        # ---- fused attention per query chunk ----
        def emit_sim(qch, g2, jt, qlo, qhi, kind, ct=None):
            """sim matmuls + exp(scale=1/||k||) + bias/mask mult -> ptx."""
            w = qhi - qlo
            base = 64 * g2
            lhs = knb_sb
            q0 = qch * QW
            w4 = 4 * w
            ps = psS.tile([128, 1024], f32, tag="S",
                          name=f"sps{qch}{g2}{jt}{kind}")
            qv = qn_sb[base:base + 64, :, :, :].rearrange(
                "p c x f -> p (c x f)")
            if kind == "x":
                off, bw = BIAS_OFFS[(qch, g2, ct)]
                assert bw == w
                bt = biap.tile([128, 1024], bf16, tag="bias",
                               name=f"bt{qch}{g2}{jt}")
                nc.sync.dma_start(
                    bt[:, 0:w4],
                    biasF[off:off + 128 * w4].rearrange("(p x) -> p x", p=128))
                add = bt
            else:
                add = corner_sb if kind == "corner" else bandm_sb
            for lo in range(0, w4, 512):
                hi = min(w4, lo + 512)
                nc.tensor.matmul(
                    ps[:, lo:hi],
                    lhsT=lhs[base:base + 64, jt * 128:(jt + 1) * 128],
                    rhs=qv[:, 4 * qlo + lo:4 * qlo + hi],
                    start=True, stop=False)
            for lo in range(0, w4, 512):
                hi = min(w4, lo + 512)
                nc.tensor.matmul(ps[:, lo:hi], lhsT=id128_sb[:],
                                 rhs=add[:, lo:hi], start=False, stop=True)
            pt = ptxp.tile([128, 1024], bf16, tag="ptx",
                           name=f"ptx{qch}{g2}{jt}{kind}")
            nc.scalar.activation(pt[:, 0:w4], ps[:, 0:w4], Exp)
            return pt

        def emit_av(avps, q0, jt, qlo, qhi, pt, start, stop, pt0=None):
            c0 = 4 * (qlo - q0)
            pt0 = 0 if pt0 is None else 4 * pt0
            cuts = sorted({c0, 4 * (qhi - q0)}
                          | {b for b in (512,) if c0 < b < 4 * (qhi - q0)})
            for lo, hi in zip(cuts[:-1], cuts[1:]):
                nc.tensor.matmul(avps[:, lo:hi],
                                 lhsT=va_sb[:, jt, :],
                                 rhs=pt[:, pt0 + lo - c0:pt0 + hi - c0],
                                 start=start, stop=stop)

        def emit_attnorm(avps, qch, g2):
            q0 = qch * QW
            lnd = rcbp.tile([64, 1024], f32, tag="lnd",
                            name=f"lnd{qch}{g2}")
            nc.scalar.activation(lnd[:], avps[64:128, :], Ln)
            rb = rcbp.tile([64, 1024], f32, tag="rcb",
                           name=f"rcb{qch}{g2}")
            nc.scalar.activation(rb[:], lnd[:], Exp, scale=-1.0)
            nc.vector.tensor_mul(
                att_sb[64 * g2:64 * g2 + 64, :, q0:q0 + QW],
                avps[0:64, :].rearrange("p (x f) -> p f x", f=4),
                rb[:].rearrange("p (x f) -> p f x", f=4))

        def emit_outproj(qc, ets=range(8)):
            for et in ets:
                ops = psS.tile([128, 1024], f32, tag="S", name=f"op{qc}{et}")
                for ftile in range(4):
                    nc.tensor.matmul(
                        ops[:, 0:512],
                        lhsT=wo_sb[:, ftile, et * 128:(et + 1) * 128],
                        rhs=att_sb[:, ftile, qc * 512:(qc + 1) * 512],
                        start=(ftile == 0), stop=(ftile == 3))
                o = osbp.tile([128, 512], bf16, tag="osb", name=f"o{qc}{et}")
                nc.vector.tensor_copy(out=o[:], in_=ops[:, 0:512])
                nc.gpsimd.dma_start(
                    out=outT[et * 128:(et + 1) * 128,
                             qc * 512:(qc + 1) * 512],
                    in_=o[:])

        deferred = {0: [], 1: []}
        order = [3, 2, 1, 0]
        for oi, qch in enumerate(order):
            q0 = qch * QW
            for g2 in range(2):
                avps = None
                xs = _x_units(qch)
                units = ([("x",) + xs[0]]
                         + [("b", jt, lo, hi, kk)
                            for jt, lo, hi, kk in _band_units(qch)]
                         + [("x",) + u for u in xs[1:]])
                pend = None
                for i, u in enumerate(units):
                    if u[0] == "x":
                        _, ct, qlo, qhi = u
                        jt, kind = 8 + ct, "x"
                        pt = emit_sim(qch, g2, jt, qlo, qhi, kind, ct=ct)
                    else:
                        _, jt, qlo, qhi, kind = u
                        pt = emit_sim(qch, g2, jt, qlo, qhi, kind)
                    # run the previous chunk's normalize/out-proj behind this
                    # group's first sim so the normalize chain and the pool
                    # WAR on the av tile are off the PE critical path
                    if i == 0 and deferred[g2]:
                        for fn in deferred[g2]:
                            fn()
                        deferred[g2] = []
                    if oi == 3 and g2 == 1 and i == 0:
                        for fn in deferred[0]:
                            fn()
                        deferred[0] = []
                    if pend is not None:
                        if avps is None:
                            avps = psA.tile([128, 1024], f32, tag="avg",
                                            name=f"av{qch}{g2}")
                        emit_av(avps, q0, *pend, start=(i == 1), stop=False)
                    pend = (jt, qlo, qhi, pt)
                emit_av(avps, q0, *pend, start=False, stop=True)
                deferred[g2] = [
                    lambda a=avps, q=qch, g=g2: emit_attnorm(a, q, g)]
                if oi == 1 and g2 == 1:
                    deferred[g2].append(lambda: emit_outproj(1, range(0, 4)))
                if oi == 2 and g2 == 0:
                    deferred[g2].append(lambda: emit_outproj(1, range(4, 8)))
        for g2 in range(2):
            for fn in deferred[g2]:
                fn()
        emit_outproj(0)

    return nc


_NC = None


def _get_nc():
    global _NC
    if _NC is None:
        _NC = _build_nc()
    return _NC


def _prep_in_maps(x, prefix_context, attn_bias, gamma, Wq, Wkv, q_scale,
                  k_scale, Wo, mask):
    import ml_dtypes
    bf = ml_dtypes.bfloat16

    x = np.asarray(x, np.float32)
    prefix_context = np.asarray(prefix_context, np.float32)
    attn_bias = np.asarray(attn_bias, np.float32)
    gamma = np.asarray(gamma, np.float32)
    Wq = np.asarray(Wq, np.float32)
    Wkv = np.asarray(Wkv, np.float32)
    q_scale = np.asarray(q_scale, np.float32)
    k_scale = np.asarray(k_scale, np.float32)
    Wo = np.asarray(Wo, np.float32)
    mask = np.asarray(mask)

    tril = np.triu(np.ones((N, N), np.float32))  # keep key c <= query i [c, i]
    sdk = (8.0 * q_scale * k_scale).astype(np.float32)
    sdkc = np.tile(sdk, 2).reshape(128, 1)
    wkvT = np.ascontiguousarray(
        Wkv.T.reshape(8, 128, 2 * DH).transpose(1, 0, 2).reshape(
            128, 8 * 2 * DH)).astype(bf)

    in_maps = []
    for c in CORES:
        b, g = c // 2, c % 2
        hs = slice(g * HL, (g + 1) * HL)
        # additive bias with causal/key-mask kills at -1e30, [h, c, i]
        eb = np.where(tril[None] > 0, attn_bias[hs].transpose(0, 2, 1),
                      -1e30)
        maskf = mask[b]
        if not maskf.all():
            eb[:, ~maskf, :] = -1e30
        eb = eb.astype(bf)
        # pack bias tiles in kernel consumption order
        bflat = np.empty(BIAS_TOTAL, bf)
        for qch in range(NQC):
            for g2 in range(2):
                heads = [2 * ft + g2 for ft in range(4)]
                for ct, qlo, qhi in _x_units(qch):
                    off, w = BIAS_OFFS[(qch, g2, ct)]
                    t = eb[heads, 128 * ct:128 * (ct + 1), qlo:qhi]
                    bflat[off:off + 128 * 4 * w] = (
                        t.transpose(1, 2, 0).ravel())
        Wg = Wq[g * FL:(g + 1) * FL] * gamma[None, :]
        s = Wg.sum(axis=1)
        wcT = Wg.T - s[None, :] / DIM
        wog = Wo[:, g * FL:(g + 1) * FL]
        def pmaj(a, nt):
            # [nt*128, m] -> [128, nt*m] partition-major
            m = a.shape[1]
            return np.ascontiguousarray(
                a.reshape(nt, 128, m).transpose(1, 0, 2).reshape(128, nt * m))

        def qbmaj(a):
            # [1024, 1024] -> [128, (qb, kt, 256)]
            return np.ascontiguousarray(
                a.reshape(8, 128, 4, 256).transpose(1, 2, 0, 3).reshape(
                    128, 8192))

        wcf = np.ascontiguousarray(
            wcT.reshape(8, 128, 4, 128).transpose(1, 2, 0, 3).reshape(
                128, 4096))
        in_maps.append(dict(
            xT=qbmaj(x[b].T).astype(bf),
            ctxT=qbmaj(prefix_context[b].T).astype(bf),
            biasF=bflat,
            wc=wcf.astype(bf),
            wkv=wkvT,
            wo=pmaj(wog.T, 4).astype(bf),
            sdkc=sdkc,
        ))
    return in_maps


def kernel(**inputs):
    from concourse.bass_utils import run_bass_kernel_spmd

    nc = _get_nc()
    in_maps = _prep_in_maps(**inputs)
    res = run_bass_kernel_spmd(nc, in_maps, CORES).results
    out = np.empty((B, N, DIM), np.float32)
    for b in range(B):
        out[b] = (np.asarray(res[2 * b]["outT"]).astype(np.float32)
                  + np.asarray(res[2 * b + 1]["outT"]).astype(np.float32)).T
    return out
